# revision 7
# baseline (speedup 1.0000x reference)
"""GCN layer (gather + segment_sum + linear + relu) on 8 trn2 NeuronCores.

Sharding: nodes and their in-edges (grouped by dst) are split across the 8
cores; core k owns dst nodes [k*12500, (k+1)*12500). The full feature table is
replicated to every core's DRAM; W/b are replicated. Per core:

  1. Edges are sorted by (src window, dst super-tile). Src windows are 4
     ranges of 25000 rows (dma_gather idx is int16, so one gather call
     addresses one window); dst super-tiles are 512 nodes wide. Edges of a
     (window, super-tile) group are padded to whole 128-edge columns; the
     column counts form a schedule shared by all 8 cores (max over cores) so
     a single Bass module serves every core.
  2. dma_gather fetches each column's 128 src rows ([128 edge-partitions x
     128 feat]) from DRAM (single SWDGE queue; multi-queue measured slower).
  3. Per column, a one-hot matrix S[e, d] = (iota[d] == dst_rel[e]) is built
     on DVE ([128 x 512]); PE accumulates h^T[f, d] += G_col.T @ S into one
     PSUM bank per group, then the group is added into a per-super-tile h^T
     SBUF accumulator. Padded slots carry dst_rel = -1 (zero S row).
  4. Per 128-node tile: out = relu(h^T_slice.T @ W + ones.T @ b) via two
     PSUM-accumulated matmuls (K=1 ones x b adds the bias) + ReLU on ScalarE,
     stored as [12544, 128] rows in natural node order.

The repeat parameter (timing only) runs the whole pipeline R times so the
per-iteration HW time can be measured as a wall-clock difference.
"""

import os

import numpy as np

import concourse.bacc as bacc
import concourse.mybir as mybir
import concourse.tile as tile
from concourse import bass_utils

P = 128
D = 128
F = 128
N_NODES = 100000
N_CORES = 8
NPC = N_NODES // N_CORES            # 12500
NPC_PAD = ((NPC + P - 1) // P) * P  # 12544
N_TILES = NPC_PAD // P              # 98
N_WIN = 4
WIN_ROWS = N_NODES // N_WIN         # 25000 (< int16 max)
SUP = 512                           # dst super-tile width (PSUM bank)
N_SUP = (NPC_PAD + SUP - 1) // SUP  # 25 (last one 256 wide)

CALL_COLS = 32    # gather-call size in 128-edge columns (4096 idxs)
N_QUEUES = 1      # multi-queue SWDGE measured ~5x slower; keep one queue


def _sup_width(ts):
    return min(SUP, NPC_PAD - ts * SUP)


def _build_schedule(edge_src, edge_dst):
    """Shared column schedule + per-core index/dst streams."""
    core_of = edge_dst // NPC
    counts = np.zeros((N_CORES, N_WIN, N_SUP), np.int64)
    per_core_raw = []
    for k in range(N_CORES):
        m = core_of == k
        dstl = (edge_dst[m] - k * NPC).astype(np.int64)
        src = edge_src[m].astype(np.int64)
        w = src // WIN_ROWS
        t = dstl // SUP
        np.add.at(counts[k], (w, t), 1)
        per_core_raw.append((dstl, src, w, t))

    ncols = (counts.max(axis=0) + P - 1) // P      # [N_WIN, N_SUP]
    tile_tot = ncols.sum(axis=0)
    ncols[0] = np.where(tile_tot == 0, 1, ncols[0])

    flat = ncols.reshape(-1)
    off_flat = np.concatenate([[0], np.cumsum(flat)])
    col_off = off_flat[:-1].reshape(N_WIN, N_SUP)
    total_cols = int(off_flat[-1])

    calls = []  # (window, col_start, col_end)
    for w in range(N_WIN):
        cur = int(col_off[w, 0])
        for t in range(N_SUP):
            ct = int(ncols[w, t])
            here = int(col_off[w, t])
            if here + ct - cur > CALL_COLS and here > cur:
                calls.append((w, cur, here))
                cur = here
        end = int(col_off[w, N_SUP - 1] + ncols[w, N_SUP - 1])
        if end > cur:
            calls.append((w, cur, end))

    per_core = []
    for k in range(N_CORES):
        dstl, src, w, t = per_core_raw[k]
        key = w * N_SUP + t
        order = np.argsort(key, kind="stable")
        key_s = key[order]
        grp_start = np.concatenate([[0], np.cumsum(np.bincount(
            key_s, minlength=N_WIN * N_SUP))])[:-1]
        pos_in_grp = np.arange(key_s.size) - grp_start[key_s]
        flatpos = off_flat[key_s] * P + pos_in_grp

        gidx = np.zeros(total_cols * P, np.int16)
        drel = np.full(total_cols * P, -1.0, np.float32)
        gidx[flatpos] = (src[order] - w[order] * WIN_ROWS).astype(np.int16)
        drel[flatpos] = (dstl[order] - t[order] * SUP).astype(np.float32)

        idx_pm = np.zeros((P, total_cols * 8), np.int16)
        for (_w, c0, c1) in calls:
            seg = gidx[c0 * P:c1 * P]
            idx_pm[:, c0 * 8:c1 * 8] = np.tile(seg.reshape(-1, 16).T, (8, 1))
        drel_pm = np.ascontiguousarray(drel.reshape(total_cols, P).T)
        per_core.append((idx_pm, drel_pm))

    return ncols, col_off, total_cols, calls, per_core


def _build_module(ncols, col_off, total_cols, calls, repeat=1):
    f32 = mybir.dt.float32
    i16 = mybir.dt.int16
    nc = bacc.Bacc(
        "TRN2", target_bir_lowering=False, debug=False,
        num_devices=N_CORES, num_swdge_queues=max(N_QUEUES, 1),
    )
    feats = nc.dram_tensor("features", [N_NODES, D], f32, kind="ExternalInput")
    ell = nc.dram_tensor("ell_idx", [P, total_cols * 8], i16,
                         kind="ExternalInput")
    drel_d = nc.dram_tensor("dstrel", [P, total_cols], f32,
                            kind="ExternalInput")
    iota_d = nc.dram_tensor("iota", [P, SUP], f32, kind="ExternalInput")
    ones_d = nc.dram_tensor("ones", [1, P], f32, kind="ExternalInput")
    w_d = nc.dram_tensor("W", [D, F], f32, kind="ExternalInput")
    b_d = nc.dram_tensor("b", [1, F], f32, kind="ExternalInput")
    out_d = nc.dram_tensor("out", [NPC_PAD, F], f32, kind="ExternalOutput")
    out_v = out_d[:].rearrange("(t p) f -> t p f", p=P)

    def call_groups(w, c0, c1):
        groups = []
        for t in range(N_SUP):
            s = max(int(col_off[w, t]), c0)
            e = min(int(col_off[w, t] + ncols[w, t]), c1)
            if e > s:
                groups.append((t, list(range(s, e))))
        return groups

    with tile.TileContext(nc) as tc:
        with (
            tc.tile_pool(name="const", bufs=1) as cpool,
            tc.tile_pool(name="ht", bufs=1) as htpool,
            tc.tile_pool(name="G", bufs=2) as gpool,
            tc.tile_pool(name="S", bufs=6) as spool,
            tc.tile_pool(name="stage", bufs=2) as stpool,
            tc.tile_pool(name="hps", bufs=4, space="PSUM") as hps,
            tc.tile_pool(name="ops", bufs=2, space="PSUM") as ops,
        ):
            idx_sb = cpool.tile([P, total_cols * 8], i16)
            nc.sync.dma_start(out=idx_sb[:], in_=ell[:])
            drel_sb = cpool.tile([P, total_cols], f32)
            nc.sync.dma_start(out=drel_sb[:], in_=drel_d[:])
            iota_sb = cpool.tile([P, SUP], f32)
            nc.sync.dma_start(out=iota_sb[:], in_=iota_d[:])
            ones_sb = cpool.tile([1, P], f32)
            nc.sync.dma_start(out=ones_sb[:], in_=ones_d[:])
            w_sb = cpool.tile([D, F], f32)
            nc.sync.dma_start(out=w_sb[:], in_=w_d[:])
            b_sb = cpool.tile([1, F], f32)
            nc.sync.dma_start(out=b_sb[:], in_=b_d[:])

            for rep in range(repeat):
                htile = {}
                for ci, (w, c0, c1) in enumerate(calls):
                    cc = c1 - c0
                    g = gpool.tile([P, cc * D], f32, tag=f"G{ci % 2}",
                                   name=f"g_{rep}_{ci}")
                    nc.gpsimd.dma_gather(
                        out_ap=g[:].rearrange("p (c d) -> p c d", d=D),
                        in_ap=feats[w * WIN_ROWS:(w + 1) * WIN_ROWS, :],
                        idxs_ap=idx_sb[:, c0 * 8:c1 * 8],
                        num_idxs=cc * P,
                        num_idxs_reg=cc * P,
                        elem_size=D,
                        single_packet=False,
                        queue_num=ci % max(N_QUEUES, 1),
                    )
                    for t, cols in call_groups(w, c0, c1):
                        sw = _sup_width(t)
                        acc = hps.tile([P, SUP], mybir.dt.float32, tag="hps",
                                       name=f"acc_{rep}_{w}_{t}")
                        for j, c in enumerate(cols):
                            s = spool.tile([P, SUP], f32, tag="S",
                                           name=f"s_{rep}_{c}")
                            nc.vector.tensor_scalar(
                                out=s[:, :sw], in0=iota_sb[:, :sw],
                                scalar1=drel_sb[:, c:c + 1], scalar2=None,
                                op0=mybir.AluOpType.is_equal,
                            )
                            nc.tensor.matmul(
                                out=acc[:, :sw],
                                lhsT=g[:, (c - c0) * D:(c - c0 + 1) * D],
                                rhs=s[:, :sw],
                                start=(j == 0),
                                stop=(j == len(cols) - 1),
                            )
                        if t not in htile:
                            htile[t] = htpool.tile(
                                [P, SUP], f32, tag=f"ht{t}", name=f"ht{t}")
                            nc.scalar.activation(
                                out=htile[t][:, :sw], in_=acc[:, :sw],
                                func=mybir.ActivationFunctionType.Copy,
                            )
                        else:
                            nc.vector.tensor_tensor(
                                out=htile[t][:, :sw], in0=htile[t][:, :sw],
                                in1=acc[:, :sw], op=mybir.AluOpType.add,
                            )

                for t in range(N_TILES):
                    ts, o = t * P // SUP, (t * P) % SUP
                    o_ps = ops.tile([P, F], mybir.dt.float32, tag="ops",
                                    name=f"ops_{rep}_{t}")
                    nc.tensor.matmul(out=o_ps[:],
                                     lhsT=htile[ts][:, o:o + P], rhs=w_sb[:],
                                     start=True, stop=False)
                    nc.tensor.matmul(out=o_ps[:], lhsT=ones_sb[:], rhs=b_sb[:],
                                     start=False, stop=True)
                    stage = stpool.tile([P, F], f32, tag="stage",
                                        name=f"st_{rep}_{t}")
                    nc.scalar.activation(
                        out=stage[:], in_=o_ps[:],
                        func=mybir.ActivationFunctionType.Relu,
                    )
                    nc.sync.dma_start(out=out_v[t], in_=stage[:])
    nc.compile()
    return nc


_CACHE: dict = {}


def _get_module(edge_src, edge_dst, repeat=1):
    key = (hash((edge_src.tobytes(), edge_dst.tobytes())), repeat)
    if _CACHE.get("key_" + str(repeat)) == key:
        return _CACHE["val_" + str(repeat)]
    if _CACHE.get("sched_key") == key[0]:
        sched = _CACHE["sched"]
    else:
        sched = _build_schedule(edge_src, edge_dst)
        _CACHE["sched_key"] = key[0]
        _CACHE["sched"] = sched
    ncols, col_off, total_cols, calls, per_core = sched
    nc = _build_module(ncols, col_off, total_cols, calls, repeat=repeat)
    _CACHE["key_" + str(repeat)] = key
    _CACHE["val_" + str(repeat)] = (nc, per_core)
    return _CACHE["val_" + str(repeat)]


def _in_maps(features, W, b, per_core):
    iota = np.ascontiguousarray(
        np.broadcast_to(np.arange(SUP, dtype=np.float32), (P, SUP)))
    ones = np.ones((1, P), np.float32)
    maps = []
    for k in range(N_CORES):
        idx_pm, drel_pm = per_core[k]
        maps.append({
            "features": features,
            "ell_idx": idx_pm,
            "dstrel": drel_pm,
            "iota": iota,
            "ones": ones,
            "W": W,
            "b": b,
        })
    return maps


def kernel(features, W, b, edge_src, edge_dst):
    features = np.ascontiguousarray(np.asarray(features), dtype=np.float32)
    W = np.ascontiguousarray(np.asarray(W), dtype=np.float32)
    b = np.ascontiguousarray(np.asarray(b), dtype=np.float32).reshape(1, F)
    edge_src = np.asarray(edge_src).astype(np.int64)
    edge_dst = np.asarray(edge_dst).astype(np.int64)

    repeat = int(os.environ.get("GCN_REPEAT", "1"))
    nc, per_core = _get_module(edge_src, edge_dst, repeat=repeat)

    res = bass_utils.run_bass_kernel_spmd(
        nc, _in_maps(features, W, b, per_core),
        core_ids=list(range(N_CORES)),
        trace=bool(int(os.environ.get("GCN_TRACE", "0"))),
    )
    if res.exec_time_ns is not None:
        print(f"HW exec time: {res.exec_time_ns} ns")

    out = np.empty((N_NODES, F), np.float32)
    for k in range(N_CORES):
        out[k * NPC:(k + 1) * NPC] = res.results[k]["out"][:NPC]
    return out
